# revision 2
# baseline (speedup 1.0000x reference)
"""Trainium2 Bass kernel for nn_CustomClassificationLoss_48765058678812.

Loss (see reference): per sample b with target t, for each class c at circular
distance d(c,t) = min((c-t)%360, (t-c)%360):
    contribution = |0.98**d - x[b,c]|  (d=0 uses 1-x, valid since x in [0,1)),
    except d == 180 contributes 0.
loss = sum of all contributions / B.

Design (pure data parallel, 8 cores x 8192 samples):
  - Host builds a [360, 360] fp16 table: row t = masked profile (0 at the
    d==180 slot) and per-core [128, 64] int32 index tiles (targets laid out
    sample b = g*128 + p  ->  [p, g]).
  - Device, per chunk of 8 groups (1024 samples):
      * gpsimd cast-DMA loads logits f32->f16 into [128, 8, 360]
      * 8x indirect_dma_start gathers profile rows (one row per partition)
      * DVE: diff = P - X ; mask = min(P*2^14, 1) ; diff *= mask   (fp16 2x/4x)
      * ACT: Abs activation with fused accumulate -> per-partition partial sums
  - Host sums the 8 x [128, 8] partials, divides by B.

Two workarounds for the pinned neuronxcc build (which allows at most ONE
sem-wait per instruction and rejects the EVENT_SEMAPHORE_RANGE_CLEAR ISA blob):
  - clear_and_free_semaphores: skip the gpsimd dma_reset/sem_clear tail ops
    (bookkeeping kept; NRT resets sems per execution - validated by repeat runs)
  - _split_multi_waits: post-pass hoisting extra sem-waits onto injected NoOps
"""

import numpy as np
from contextlib import ExitStack

import concourse.bass as bass
import concourse.tile as tile
from concourse import mybir
from concourse.bass_utils import run_bass_kernel_spmd

NUM_CLASSES = 360
DECAY = 0.98
N_CORES = 8
B_TOTAL = 65536
B_SHARD = B_TOTAL // N_CORES        # 8192
GROUPS = B_SHARD // 128             # 64 groups of 128 samples
CHUNK_GROUPS = 8                    # groups per processing chunk
NCHUNK = GROUPS // CHUNK_GROUPS     # 8 chunks
MASK_SCALE = 16384.0                # min(P*scale, 1): 0 stays 0, w>=0.027 -> 1

_CACHE: dict = {}


def _patched_clear_and_free_semaphores(self, sems):
    # The pinned walrus rejects the EVENT_SEMAPHORE_RANGE_CLEAR InstISA the
    # stock implementation emits; keep only the allocator bookkeeping.
    if not sems:
        return
    sem_nums = [s.num if hasattr(s, "num") else s for s in sems]
    self._state.prepend_free_semaphores(sem_nums)
    for poison_set in self._tile_sem_poison_stack:
        poison_set.update(sem_nums)


def _split_multi_waits(nc):
    # The pinned walrus accepts at most one sem-wait per instruction; hoist
    # extras onto same-engine NoOps placed immediately before.
    for f in nc.m.functions:
        for b in f.blocks:
            out = []
            changed = False
            for ins in b.instructions:
                si = ins.sync_info
                waits = list(si.on_wait) if (si and si.on_wait) else []
                if len(waits) > 1 and ins.engine is not None:
                    for j, w in enumerate(waits[:-1]):
                        nop = mybir.InstNoOp(
                            name=f"{ins.name}_hw{j}", engine=ins.engine,
                            ins=[], outs=[],
                        )
                        nop.sync_info = mybir.SyncInfo(on_wait=[w], on_update=[])
                        nc.register_instruction(nop)
                        out.append(nop)
                    si.on_wait = [waits[-1]]
                    changed = True
                out.append(ins)
            if changed:
                b.instructions = out


def _build_table() -> np.ndarray:
    c = np.arange(NUM_CLASSES)
    o = (c[None, :] - c[:, None]) % NUM_CLASSES
    d = np.minimum(o, NUM_CLASSES - o)
    prof = DECAY ** d.astype(np.float64)
    prof[d == 180] = 0.0
    return prof.astype(np.float16)


def _build_nc() -> bass.Bass:
    bass.Bass.clear_and_free_semaphores = _patched_clear_and_free_semaphores
    nc = bass.Bass()
    f16 = mybir.dt.float16
    f32 = mybir.dt.float32
    logits = nc.dram_tensor(
        "logits", [B_SHARD, NUM_CLASSES], f32, kind="ExternalInput"
    )
    tab = nc.dram_tensor("tab", [NUM_CLASSES, NUM_CLASSES], f16, kind="ExternalInput")
    tidx = nc.dram_tensor("tidx", [128, GROUPS], mybir.dt.int32, kind="ExternalInput")
    partial = nc.dram_tensor("partial", [128, NCHUNK], f32, kind="ExternalOutput")

    # sample b = n*128 + p -> partition p, group n
    logits_r = logits.rearrange("(n p) c -> p n c", p=128)

    with tile.TileContext(nc) as tc, ExitStack() as ctx:
        singles = ctx.enter_context(tc.tile_pool(name="singles", bufs=1))
        xpool = ctx.enter_context(tc.tile_pool(name="xpool", bufs=3))
        ppool = ctx.enter_context(tc.tile_pool(name="ppool", bufs=3))
        dpool = ctx.enter_context(tc.tile_pool(name="dpool", bufs=2))
        mpool = ctx.enter_context(tc.tile_pool(name="mpool", bufs=2))
        apool = ctx.enter_context(tc.tile_pool(name="apool", bufs=2))

        idx_sb = singles.tile([128, GROUPS], mybir.dt.int32)
        nc.sync.dma_start(out=idx_sb, in_=tidx[:, :])
        accbuf = singles.tile([128, NCHUNK], mybir.dt.float32)

        for i in range(NCHUNK):
            g0 = i * CHUNK_GROUPS
            xt = xpool.tile([128, CHUNK_GROUPS, NUM_CLASSES], f16, tag="xt")
            nc.gpsimd.dma_start(
                out=xt, in_=logits_r[:, g0 : g0 + CHUNK_GROUPS, :]
            )
            pt = ppool.tile([128, CHUNK_GROUPS, NUM_CLASSES], f16, tag="pt")
            for g in range(CHUNK_GROUPS):
                nc.gpsimd.indirect_dma_start(
                    out=pt[:, g, :],
                    out_offset=None,
                    in_=tab[:, :],
                    in_offset=bass.IndirectOffsetOnAxis(
                        ap=idx_sb[:, g0 + g : g0 + g + 1], axis=0
                    ),
                )
            dt_ = dpool.tile([128, CHUNK_GROUPS, NUM_CLASSES], f16, tag="dt")
            nc.vector.tensor_sub(out=dt_, in0=pt, in1=xt)
            mt = mpool.tile([128, CHUNK_GROUPS, NUM_CLASSES], f16, tag="mt")
            nc.vector.tensor_scalar(
                out=mt,
                in0=pt,
                scalar1=MASK_SCALE,
                scalar2=1.0,
                op0=mybir.AluOpType.mult,
                op1=mybir.AluOpType.min,
            )
            nc.vector.tensor_mul(out=dt_, in0=dt_, in1=mt)
            at = apool.tile([128, CHUNK_GROUPS, NUM_CLASSES], f16, tag="at")
            nc.scalar.activation(
                out=at,
                in_=dt_,
                func=mybir.ActivationFunctionType.Abs,
                accum_out=accbuf[:, i : i + 1],
            )

        nc.sync.dma_start(out=partial[:, :], in_=accbuf)

    _split_multi_waits(nc)
    nc.finalize()
    return nc


def _get_nc() -> bass.Bass:
    if "nc" not in _CACHE:
        _CACHE["nc"] = _build_nc()
    return _CACHE["nc"]


def _prep_in_maps(logits: np.ndarray, targets: np.ndarray) -> list[dict]:
    tab = _CACHE.get("tab")
    if tab is None:
        tab = _CACHE["tab"] = _build_table()
    in_maps = []
    for core in range(N_CORES):
        sl = slice(core * B_SHARD, (core + 1) * B_SHARD)
        t = np.ascontiguousarray(targets[sl]).astype(np.int32)
        # sample b = g*128 + p -> idx[p, g]
        idx = np.ascontiguousarray(t.reshape(GROUPS, 128).T)
        in_maps.append(
            {
                "logits": np.ascontiguousarray(logits[sl]),
                "tab": tab,
                "tidx": idx,
            }
        )
    return in_maps


def kernel(logits, targets):
    logits = np.asarray(logits, dtype=np.float32)
    targets_np = np.asarray(targets).astype(np.int64)
    assert logits.shape == (B_TOTAL, NUM_CLASSES), logits.shape
    assert targets_np.shape == (B_TOTAL,), targets_np.shape

    nc = _get_nc()
    in_maps = _prep_in_maps(logits, targets_np)
    res = run_bass_kernel_spmd(nc, in_maps, core_ids=list(range(N_CORES)))
    total = np.float64(0.0)
    for out_map in res.results:
        total += np.asarray(out_map["partial"], np.float64).sum()
    loss = np.float32(total / B_TOTAL)
    return (loss, 0.0, loss)
